# revision 35
# baseline (speedup 1.0000x reference)
"""Trainium2 Bass kernel for nn_Critic (MLP value function + GAE).

Sharding: batch B=2048 split across 8 NeuronCores (256 each). MLP params
replicated. The time recurrence (reverse GAE scan) is independent per batch
element, so no cross-core communication.

Per-core strategy (v4):
  - states transposed to feature-major and cast to bf16 on the HOST; all
    device tensors are laid out so every load is ONE contiguous DMA (the
    Sync engine serializes DMA triggers at ~600ns each, so DMA COUNT -- not
    bytes -- dominated the old startup critical path).
  - single-pass bf16 matmuls (rel-err ~5e-3, gate 2e-2): weights stationary,
    activations moving with N=512 free dim -> 1 cycle/row at 2.4 GHz.
  - chunk 0 layer 0 runs k-outer in two 4-m-tile waves over quarter-loaded
    states/W0 so the PE starts ~5us in instead of waiting for all of W0.
  - ELU(z) = min(exp(z)-1, relu(z)): ScalarE Exp (+bias, from PSUM) +
    VectorE relu (+bias) + VectorE combine downcasting straight to bf16.
    A dummy Exp at t=0 preloads the ACT function table.
  - value head: Wo stationary (1-column LDWEIGHTS ~ free), h3 moving ->
    pv [1, cols] PSUM row; +bo fused into ACT copies into a [1, NCOLS]
    value row (time-reversed segments); one rearranged SBUF->SBUF DMA
    scatters it into valT2 [128 batch, 2*(T+1)] at the end.
  - GAE: wide [128, 2*T] VectorE ops over both batch blocks at once; the
    reverse scan is one tensor_tensor_scan with a zeroed multiplier at the
    second block's first column (scan-reset trick).
"""

import sys

sys.path.insert(0, "/opt/trn_rl_repo")

import numpy as np

T, B, D, H = 16, 2048, 2048, 1024
NCORES = 8
BC = B // NCORES  # 256 batch per core
TP1 = T + 1
DISCOUNT, LAMBDA = 0.99, 0.95
P = 128
KD = D // P  # 16 k-tiles for layer 0
KH = H // P  # 8 k-tiles for layers 1,2,out
MH = H // P  # 8 m-tiles of hidden units
NCOLS = TP1 * BC  # 4352 moving columns per core
CHUNK = 512  # moving columns per chunk (2 time steps)

_NC_CACHE = None


def _build():
    import concourse.bacc as bacc
    import concourse.mybir as mybir
    from concourse.tile import TileContext

    F32 = mybir.dt.float32
    BF16 = mybir.dt.bfloat16
    ALU = mybir.AluOpType
    ACTF = mybir.ActivationFunctionType

    nc = bacc.Bacc(None, target_bir_lowering=False, debug=False)

    statesZ_h = nc.declare_dram_parameter("statesZ", [P, KD, NCOLS], BF16, isOutput=False)
    rew_h = nc.declare_dram_parameter("rew_rev", [BC, T], F32, isOutput=False)
    cont_h = nc.declare_dram_parameter("cont_rev", [BC, TP1], F32, isOutput=False)
    w0_h = nc.declare_dram_parameter("W0z", [P, KD * H], BF16, isOutput=False)
    b0_h = nc.declare_dram_parameter("b0", [P, MH], F32, isOutput=False)
    w1_h = nc.declare_dram_parameter("W1z", [P, KH * H], BF16, isOutput=False)
    b1_h = nc.declare_dram_parameter("b1", [P, MH], F32, isOutput=False)
    w2_h = nc.declare_dram_parameter("W2z", [P, KH * H], BF16, isOutput=False)
    b2_h = nc.declare_dram_parameter("b2", [P, MH], F32, isOutput=False)
    wo_h = nc.declare_dram_parameter("Woz", [P, KH], BF16, isOutput=False)
    bo_h = nc.declare_dram_parameter("bo", [P, 1], F32, isOutput=False)
    ret_h = nc.declare_dram_parameter("ret_bt", [BC, T], F32, isOutput=True)
    val_h = nc.declare_dram_parameter("val_bt", [BC, T], F32, isOutput=True)

    with TileContext(nc) as tc:
        with (
            tc.tile_pool(name="wpool", bufs=1) as wpool,
            tc.tile_pool(name="stpool", bufs=2) as stpool,
            tc.tile_pool(name="hpool", bufs=2) as hpool,
            tc.tile_pool(name="tmp", bufs=4) as tmppool,
            tc.tile_pool(name="gae", bufs=1) as gaepool,
            tc.tile_pool(name="psA", bufs=4, space="PSUM") as psApool,
            tc.tile_pool(name="psV", bufs=2, space="PSUM") as psVpool,
        ):
            # ---- DMA issue order IS the startup critical path ----
            # chunk-0 states + W0 interleaved in quarters first, then biases
            # (layer-0 ELU needs them), then W1/W2/head/GAE inputs.
            stbig0 = stpool.tile([P, KD * CHUNK], BF16, name="st", tag="st", bufs=2)
            st0_3d = stbig0[:].rearrange("p (k c) -> p k c", k=KD, c=CHUNK)
            w0big = wpool.tile([P, KD * H], BF16, name="w0", tag="w0")
            for q in range(4):
                ks = slice(q * 4, (q + 1) * 4)
                nc.sync.dma_start(out=st0_3d[:, ks, :], in_=statesZ_h[0:P, ks, 0:CHUNK])
                nc.sync.dma_start(
                    out=w0big[:, q * 4 * H : (q + 1) * 4 * H],
                    in_=w0_h[0:P, q * 4 * H : (q + 1) * 4 * H],
                )
            bsb = []
            for li, bh in enumerate((b0_h, b1_h, b2_h)):
                bt = wpool.tile([P, MH], F32, name=f"bsb{li}", tag=f"bsb{li}")
                nc.sync.dma_start(out=bt[:], in_=bh[0:P, 0:MH])
                bsb.append(bt)
            wosb = wpool.tile([P, KH], BF16, name="wosb", tag="wosb")
            nc.sync.dma_start(out=wosb[:], in_=wo_h[0:P, 0:KH])
            wo32 = wpool.tile([P, KH], F32, name="wo32", tag="wo32")
            nc.scalar.copy(wo32[:], wosb[:])
            ones1 = wpool.tile([P, 1], BF16, name="ones1", tag="ones1")
            nc.vector.memset(ones1[:], 1.0)
            bobc = wpool.tile([P, 1], F32, name="bobc", tag="bobc")
            nc.sync.dma_start(out=bobc[:], in_=bo_h[0:P, 0:1])

            # dummy Exp: pull the ACT function table load off the critical path
            dumt = wpool.tile([1, 2], F32, name="dumt", tag="dumt")
            nc.vector.memset(dumt[:], 0.0)
            nc.scalar.activation(dumt[0:1, 0:1], dumt[0:1, 1:2], ACTF.Exp)
            # dummy matmuls bridge the initial DMA wait (~8us) with the HAM
            # clock gate warm, so the real stream starts at 2.4GHz
            dumw = wpool.tile([P, CHUNK], BF16, name="dumw", tag="dumw")
            nc.vector.memset(dumw[:], 0.0)
            psD = psVpool.tile([1, CHUNK], F32, name="psD", tag="pv")
            for _ in range(24):
                nc.tensor.matmul(
                    psD[:],
                    lhsT=dumw[:, 0:1],
                    rhs=dumw[:],
                    start=True,
                    stop=True,
                    skip_group_check=True,
                )

            w1big = wpool.tile([P, KH * H], BF16, name="w1", tag="w1")
            nc.sync.dma_start(out=w1big[:], in_=w1_h[0:P, :])
            w2big = wpool.tile([P, KH * H], BF16, name="w2", tag="w2")
            nc.sync.dma_start(out=w2big[:], in_=w2_h[0:P, :])

            # GAE inputs: [128, blk*(T+1)+rt] layout, one DMA each
            ct2 = gaepool.tile([P, 2 * TP1], F32, name="cont2", tag="cont2")
            nc.sync.dma_start(
                out=ct2[:].rearrange("p (blk rt) -> p blk rt", blk=2, rt=TP1),
                in_=cont_h[0:BC, 0:TP1].rearrange("(blk p) rt -> p blk rt", blk=2, p=P),
            )
            rw2 = gaepool.tile([P, 2 * T], F32, name="rew2", tag="rew2")
            nc.sync.dma_start(
                out=rw2[:].rearrange("p (blk t) -> p blk t", blk=2, t=T),
                in_=rew_h[0:BC, 0:T].rearrange("(blk p) t -> p blk t", blk=2, p=P),
            )

            valT2 = gaepool.tile([P, 2 * TP1], F32, name="valT2", tag="valT2")

            # disc/dl only need cont: compute them up front, off the tail
            c3 = ct2[:].rearrange("p (blk rt) -> p blk rt", blk=2, rt=TP1)
            disc = gaepool.tile([P, 2 * T], F32, name="disc", tag="disc")
            d3 = disc[:].rearrange("p (blk t) -> p blk t", blk=2, t=T)
            nc.vector.tensor_scalar_mul(d3, c3[:, :, 0:T], DISCOUNT)
            dl = gaepool.tile([P, 2 * T], F32, name="dl", tag="dl")
            nc.vector.tensor_scalar_mul(dl[:], disc[:], LAMBDA)
            # scan-reset: zero the multiplier at block 1's first column so one
            # scan does two independent reverse recurrences
            nc.vector.memset(dl[:, T : T + 1], 0.0)

            def elu_store(ps_ap, out_ap, bias_col, cols):
                # out = min(exp(z+b)-1, relu(z+b)) with z in PSUM
                e = tmppool.tile([P, CHUNK], F32, name="e", tag="e", bufs=4)
                nc.scalar.activation(e[:, :cols], ps_ap, ACTF.Exp, bias=bias_col)
                rl = tmppool.tile([P, CHUNK], F32, name="rl", tag="rl", bufs=4)
                nc.vector.tensor_scalar(
                    rl[:, :cols], ps_ap, bias_col, 0.0, ALU.add, ALU.max
                )
                nc.vector.scalar_tensor_tensor(
                    out_ap, e[:, :cols], 1.0, rl[:, :cols], ALU.subtract, ALU.min
                )

            chunks = []
            c0 = 0
            while c0 < NCOLS:
                cols = min(CHUNK, NCOLS - c0)
                chunks.append((c0, cols))
                c0 += cols

            # head finisher (PE reduction + ACT + scatter) for chunk c is
            # deferred into chunk c+1's body so the PE reaches it well after
            # the DVE accumulate chain has produced accb (no PE stall)
            pending_head = []

            def flush_head():
                while pending_head:
                    pending_head.pop(0)()

            for ci, (col0, cols) in enumerate(chunks):
                if ci == 0:
                    stbig = stbig0
                else:
                    stbig = stpool.tile(
                        [P, KD * CHUNK], BF16, name="st", tag="st", bufs=2
                    )
                    nc.sync.dma_start(
                        out=stbig[:].rearrange("p (k c) -> p k c", k=KD, c=CHUNK)[
                            :, :, :cols
                        ],
                        in_=statesZ_h[0:P, 0:KD, col0 : col0 + cols],
                    )

                # layer 0
                h1 = hpool.tile([P, MH * CHUNK], BF16, name="h1", tag="h1", bufs=2)
                if ci == 0:
                    # two 4-m-tile waves, k-outer: the first matmul needs only
                    # the first quarter of (W0, states) to be resident
                    for wave in range(2):
                        pss = [
                            psApool.tile([P, CHUNK], F32, name="ps", tag="ps")
                            for _ in range(4)
                        ]
                        for k in range(KD):
                            for mi, ps in enumerate(pss):
                                m = wave * 4 + mi
                                nc.tensor.matmul(
                                    ps[:, :cols],
                                    lhsT=w0big[:, k * H + m * P : k * H + (m + 1) * P],
                                    rhs=stbig[:, k * CHUNK : k * CHUNK + cols],
                                    start=(k == 0),
                                    stop=(k == KD - 1),
                                    skip_group_check=True,
                                )
                        for mi, ps in enumerate(pss):
                            m = wave * 4 + mi
                            elu_store(
                                ps[:, :cols],
                                h1[:, m * CHUNK : m * CHUNK + cols],
                                bsb[0][:, m : m + 1],
                                cols,
                            )
                else:
                    for m in range(MH):
                        ps = psApool.tile([P, CHUNK], F32, name="ps", tag="ps")
                        for k in range(KD):
                            nc.tensor.matmul(
                                ps[:, :cols],
                                lhsT=w0big[:, k * H + m * P : k * H + (m + 1) * P],
                                rhs=stbig[:, k * CHUNK : k * CHUNK + cols],
                                start=(k == 0),
                                stop=(k == KD - 1),
                                skip_group_check=True,
                            )
                        elu_store(
                            ps[:, :cols],
                            h1[:, m * CHUNK : m * CHUNK + cols],
                            bsb[0][:, m : m + 1],
                            cols,
                        )
                        if m == 0:
                            flush_head()

                # layers 1 and 2
                hin = h1
                for li, (wbig, bias) in enumerate(((w1big, bsb[1]), (w2big, bsb[2]))):
                    hout = hpool.tile(
                        [P, MH * CHUNK], BF16, name=f"h{li + 2}", tag=f"h{li + 2}", bufs=2
                    )
                    for m in range(MH):
                        ps = psApool.tile([P, CHUNK], F32, name="ps", tag="ps")
                        for k in range(KH):
                            nc.tensor.matmul(
                                ps[:, :cols],
                                lhsT=wbig[:, k * H + m * P : k * H + (m + 1) * P],
                                rhs=hin[:, k * CHUNK : k * CHUNK + cols],
                                start=(k == 0),
                                stop=(k == KH - 1),
                                skip_group_check=True,
                            )
                        elu_store(
                            ps[:, :cols],
                            hout[:, m * CHUNK : m * CHUNK + cols],
                            bias[:, m : m + 1],
                            cols,
                        )
                    hin = hout
                h3 = hin

                if ci < len(chunks) - 1:
                    # value head on the Vector engine (PE is the bottleneck):
                    # acc[p, c] = sum_k h3_k[p, c] * Wo[p, k], then one fp32
                    # ones-reduction matmul collapses partitions -> pv [1,cols]
                    acc = tmppool.tile([P, CHUNK], F32, name="acc", tag="acc", bufs=2)
                    nc.vector.tensor_scalar_mul(
                        acc[:, :cols], h3[:, 0:cols], wo32[:, 0:1]
                    )
                    for k in range(1, KH - 1):
                        nc.vector.scalar_tensor_tensor(
                            acc[:, :cols],
                            h3[:, k * CHUNK : k * CHUNK + cols],
                            wo32[:, k : k + 1],
                            acc[:, :cols],
                            ALU.mult,
                            ALU.add,
                        )
                    # last accumulate downcasts to bf16 so the ones-reduction
                    # matmul runs at full bf16 rate
                    accb = tmppool.tile([P, CHUNK], BF16, name="accb", tag="accb", bufs=2)
                    nc.vector.scalar_tensor_tensor(
                        accb[:, :cols],
                        h3[:, (KH - 1) * CHUNK : (KH - 1) * CHUNK + cols],
                        wo32[:, KH - 1 : KH],
                        acc[:, :cols],
                        ALU.mult,
                        ALU.add,
                    )
                    def head_finish(accb=accb, col0=col0, cols=cols):
                        pv = psVpool.tile([1, CHUNK], F32, name="pv", tag="pv")
                        nc.tensor.matmul(
                            pv[:, :cols],
                            lhsT=ones1[:],
                            rhs=accb[:, :cols],
                            start=True,
                            stop=True,
                            skip_group_check=True,
                        )
                        # +bo via ACT copy to SBUF, then tiny partition-scatter
                        # DMAs place each [1,128] segment as a valT2 column
                        # (time-reversed); the scatters overlap later chunks
                        vrow = tmppool.tile(
                            [1, CHUNK], F32, name="vrow", tag="vrow", bufs=2
                        )
                        nc.scalar.activation(
                            vrow[0:1, :cols],
                            pv[0:1, :cols],
                            ACTF.Identity,
                            bias=bobc[0:1, 0:1],
                        )
                        for s in range(cols // BC):
                            rt = TP1 - 1 - (col0 // BC + s)
                            for blk in range(2):
                                off = s * BC + blk * P
                                nc.sync.dma_start(
                                    out=valT2[:, blk * TP1 + rt : blk * TP1 + rt + 1],
                                    in_=vrow[0:1, off : off + P],
                                )

                    pending_head.append(head_finish)
                else:
                    # last chunk: h3 stationary -> pv [128 batch, 1], so the
                    # valT2 column write is a partition-aligned ACT copy (no
                    # DMA completion latency on the tail critical path)
                    for s in range(cols // BC):
                        rt = TP1 - 1 - (col0 // BC + s)
                        for blk in range(2):
                            off = s * BC + blk * P
                            pvb = psVpool.tile([P, 1], F32, name="pvb", tag="pvb")
                            for k in range(KH):
                                nc.tensor.matmul(
                                    pvb[:],
                                    lhsT=h3[:, k * CHUNK + off : k * CHUNK + off + P],
                                    rhs=wosb[:, k : k + 1],
                                    start=(k == 0),
                                    stop=(k == KH - 1),
                                    skip_group_check=True,
                                )
                            nc.scalar.activation(
                                valT2[:, blk * TP1 + rt : blk * TP1 + rt + 1],
                                pvb[:],
                                ACTF.Identity,
                                bias=bobc[:, 0:1],
                            )

            flush_head()

            # ---- GAE: wide [128, 2*T] ops over both batch blocks ----
            v3 = valT2[:].rearrange("p (blk rt) -> p blk rt", blk=2, rt=TP1)
            # val output only needs valT2: overlap it with the GAE compute
            nc.sync.dma_start(
                out=val_h[0:BC, 0:T].rearrange("(blk p) t -> p blk t", blk=2, p=P),
                in_=v3[:, :, 1:TP1],
            )
            dtt = gaepool.tile([P, 2 * T], F32, name="dtt", tag="dtt")
            t3 = dtt[:].rearrange("p (blk t) -> p blk t", blk=2, t=T)
            nc.vector.tensor_mul(t3, d3, v3[:, :, 0:T])
            nc.vector.tensor_add(dtt[:], dtt[:], rw2[:])
            nc.vector.tensor_sub(t3, t3, v3[:, :, 1:TP1])
            adv = gaepool.tile([P, 2 * T], F32, name="adv", tag="adv")
            nc.vector.tensor_tensor_scan(adv[:], dl[:], dtt[:], 0.0, ALU.mult, ALU.add)
            ret2 = gaepool.tile([P, 2 * T], F32, name="ret2", tag="ret2")
            r3 = ret2[:].rearrange("p (blk t) -> p blk t", blk=2, t=T)
            nc.vector.tensor_add(r3, adv[:].rearrange("p (blk t) -> p blk t", blk=2, t=T), v3[:, :, 1:TP1])

            nc.sync.dma_start(
                out=ret_h[0:BC, 0:T].rearrange("(blk p) t -> p blk t", blk=2, p=P),
                in_=r3,
            )

    nc.compile()
    return nc


def _get_nc():
    global _NC_CACHE
    if _NC_CACHE is None:
        _NC_CACHE = _build()
    return _NC_CACHE


def _make_in_maps(inputs):
    import ml_dtypes

    bf16 = ml_dtypes.bfloat16
    states = np.asarray(inputs["states"], dtype=np.float32)
    reward = np.asarray(inputs["reward"], dtype=np.float32)
    cont = np.asarray(inputs["cont"], dtype=np.float32)

    states_bf = states.astype(bf16)  # [TP1, B, D]

    def wz(w, nk):
        # [nk*P, H] -> [P, nk*H]: row p holds k-tile-major weight columns
        w = np.asarray(w, np.float32).astype(bf16)
        return np.ascontiguousarray(
            w.reshape(nk, P, -1).transpose(1, 0, 2).reshape(P, -1)
        )

    W0 = wz(inputs["W0"], KD)
    W1 = wz(inputs["W1"], KH)
    W2 = wz(inputs["W2"], KH)
    Wo = wz(np.asarray(inputs["Wo"], np.float32).reshape(H, 1), KH)

    def bz(b):
        return np.ascontiguousarray(
            np.asarray(b, np.float32).reshape(MH, P).T
        )

    b0, b1, b2 = bz(inputs["b0"]), bz(inputs["b1"]), bz(inputs["b2"])
    bo = np.full((P, 1), float(np.asarray(inputs["bo"], np.float32).reshape(())), np.float32)

    in_maps = []
    for c in range(NCORES):
        sl = slice(c * BC, (c + 1) * BC)
        # [TP1, BC, D] -> [P, KD, TP1*BC]: statesZ[p, k, t*BC+b]
        stT = np.transpose(states_bf[:, sl, :], (2, 0, 1)).reshape(KD, P, NCOLS)
        stZ = np.ascontiguousarray(stT.transpose(1, 0, 2))
        in_maps.append(
            {
                "statesZ": stZ,
                "rew_rev": np.ascontiguousarray(reward[::-1, sl].T),
                "cont_rev": np.ascontiguousarray(cont[::-1, sl].T),
                "W0z": W0,
                "b0": b0,
                "W1z": W1,
                "b1": b1,
                "W2z": W2,
                "b2": b2,
                "Woz": Wo,
                "bo": bo,
            }
        )
    return in_maps


def _run(inputs, trace=False):
    from concourse.bass_utils import run_bass_kernel_spmd

    nc = _get_nc()
    in_maps = _make_in_maps(inputs)
    bkr = run_bass_kernel_spmd(nc, in_maps, list(range(NCORES)), trace=trace)
    ret = np.empty((T, B), np.float32)
    val = np.empty((T, B), np.float32)
    for c in range(NCORES):
        sl = slice(c * BC, (c + 1) * BC)
        ret[:, sl] = bkr.results[c]["ret_bt"].T[::-1]
        val[:, sl] = bkr.results[c]["val_bt"].T[::-1]
    return (ret, val), bkr


def kernel(**inputs):
    out, _ = _run(inputs, trace=False)
    return out
